# revision 9
# baseline (speedup 1.0000x reference)
"""Trainium2 Bass kernel for nn_Block_25323127177341 (moe_routing).

Distribution (8 NeuronCores):
  Phase 1 (data-parallel, 512 tokens/core): fused QKV projections (fp16 PE,
    transposed-activation layout), causal multi-head attention with fp32-PSUM
    softmax (no max-subtraction; scores are small), output projection,
    residual + LayerNorm1 -> x1.
  Host: router logits / top-2 / gates / load-balance loss in exact fp32;
    all-to-all token routing done host-side (full-IO contract).
  Phase 2 (expert-parallel, 1 expert/core): gathered tokens through the
    expert MLP (fp16 PE, fp32 accum), relu, gate scaling -> contributions.
  Host: scatter-add combine + residual.
  Phase 3 (data-parallel): LayerNorm2 -> x2.
"""

import sys

import numpy as np

try:
    import concourse.bass as bass  # noqa: F401
except ImportError:  # pragma: no cover
    for _p in ("/opt/trn_rl_repo", "/root/.axon_site/_ro/trn_rl_repo"):
        if _p not in sys.path:
            sys.path.insert(0, _p)
    import concourse.bass as bass  # noqa: F401

import concourse.mybir as mybir
import concourse.tile as tile
from concourse import bacc
from concourse.bass_utils import run_bass_kernel_spmd

F16 = mybir.dt.float16
F32 = mybir.dt.float32
AF = mybir.ActivationFunctionType
ALU = mybir.AluOpType

P = 128
B, T, C, H, E, TOPK = 4, 1024, 1024, 16, 8, 2
D = C // H            # 64 head dim
HID = 4 * C           # 4096
NCORES = 8
TCORE = T // 2        # 512 tokens per core in phase 1/3
CC = C // P           # 8 C-chunks
QTILES = TCORE // P   # 4 query tiles per core
CAP = 1280            # expert capacity (max observed count 1086)
CAP_SLICE = 256
NSLICE = CAP // CAP_SLICE
HC = HID // P         # 32 hidden chunks


def _bcast_ap(vec_ap, parts=P):
    """DRAM AP for a 1-D vector replicated across `parts` partitions."""
    return bass.AP(
        tensor=vec_ap.tensor,
        offset=vec_ap.offset,
        ap=[[0, parts]] + [list(d) for d in vec_ap.ap],
    )


def _layernorm(nc, pool, y_sb, g_rep, b_rep, eps_sb, out_sb, tag):
    """out = LN(y) * g + b over the free dim (1024), tokens on partitions."""
    stats = pool.tile([P, 2, 6], F32, tag=f"lnst{tag}")
    yr = y_sb.rearrange("p (s d) -> p s d", s=2)
    for s in range(2):
        nc.vector.bn_stats(out=stats[:, s, :], in_=yr[:, s, :])
    mv = pool.tile([P, 2], F32, tag=f"lnmv{tag}")
    nc.vector.bn_aggr(out=mv, in_=stats)
    rstd = pool.tile([P, 1], F32, tag=f"lnrs{tag}")
    nc.scalar.activation(out=rstd, in_=mv[:, 1:2], func=AF.Sqrt, bias=eps_sb)
    nc.vector.reciprocal(out=rstd, in_=rstd)
    nc.vector.tensor_scalar(
        out=out_sb, in0=y_sb, scalar1=mv[:, 0:1], scalar2=rstd,
        op0=ALU.subtract, op1=ALU.mult,
    )
    nc.vector.tensor_mul(out_sb, out_sb, g_rep)
    nc.vector.tensor_add(out_sb, out_sb, b_rep)


def build_phase1():
    nc = bacc.Bacc(None)
    xqT = nc.declare_dram_parameter("xqT", [C, TCORE], F16, isOutput=False)
    xkvT = nc.declare_dram_parameter("xkvT", [C, T], F16, isOutput=False)
    xn = nc.declare_dram_parameter("xn", [TCORE, C], F32, isOutput=False)
    m01 = nc.declare_dram_parameter("m01", [T, TCORE], F16, isOutput=False)
    wq = nc.declare_dram_parameter("wq", [C, C], F16, isOutput=False)
    wk = nc.declare_dram_parameter("wk", [C, C], F16, isOutput=False)
    wv = nc.declare_dram_parameter("wv", [C, C], F16, isOutput=False)
    wo = nc.declare_dram_parameter("wo", [C, C], F16, isOutput=False)
    bq = nc.declare_dram_parameter("bq", [C], F32, isOutput=False)
    bk = nc.declare_dram_parameter("bk", [C], F32, isOutput=False)
    bv = nc.declare_dram_parameter("bv", [C], F32, isOutput=False)
    bo = nc.declare_dram_parameter("bo", [C], F32, isOutput=False)
    g1 = nc.declare_dram_parameter("g1", [C], F32, isOutput=False)
    bl1 = nc.declare_dram_parameter("bl1", [C], F32, isOutput=False)
    x1 = nc.declare_dram_parameter("x1", [TCORE, C], F32, isOutput=True)

    with tile.TileContext(nc) as tc:
        with tc.tile_pool(name="cn", bufs=1) as cn, \
             tc.tile_pool(name="wp", bufs=2) as wp, \
             tc.tile_pool(name="wk2", bufs=3) as wrk, \
             tc.tile_pool(name="ptp", bufs=3) as ptp, \
             tc.tile_pool(name="pss", bufs=1, space="PSUM") as pss, \
             tc.tile_pool(name="pso", bufs=2, space="PSUM") as pso, \
             tc.tile_pool(name="psb", bufs=1, space="PSUM") as psb, \
             tc.tile_pool(name="psm", bufs=2, space="PSUM") as psm:

            # ---------------- constant / input loads ----------------
            xqT_sb = cn.tile([P, CC, TCORE], F16, name="xqT_sb")
            nc.sync.dma_start(xqT_sb, xqT.rearrange("(c p) q -> p c q", p=P))
            xkvT_sb = cn.tile([P, CC, T], F16, name="xkvT_sb")
            nc.sync.dma_start(xkvT_sb, xkvT.rearrange("(c p) t -> p c t", p=P))
            m01_sb = cn.tile([P, CC, TCORE], F16, name="m01_sb")
            nc.sync.dma_start(m01_sb, m01.rearrange("(c p) q -> p c q", p=P))

            bq_sb = cn.tile([P, CC], F32, name="bq_sb")
            nc.gpsimd.dma_start(bq_sb, bq.rearrange("(o p) -> p o", p=P))
            bk_sb = cn.tile([P, CC], F32, name="bk_sb")
            nc.gpsimd.dma_start(bk_sb, bk.rearrange("(o p) -> p o", p=P))
            bv_rep = cn.tile([P, C], F32, name="bv_rep")
            nc.gpsimd.dma_start(bv_rep, _bcast_ap(bv[:]))
            bo_rep = cn.tile([P, C], F32, name="bo_rep")
            nc.gpsimd.dma_start(bo_rep, _bcast_ap(bo[:]))
            g1_rep = cn.tile([P, C], F32, name="g1_rep")
            nc.gpsimd.dma_start(g1_rep, _bcast_ap(g1[:]))
            bl1_rep = cn.tile([P, C], F32, name="bl1_rep")
            nc.gpsimd.dma_start(bl1_rep, _bcast_ap(bl1[:]))
            eps_sb = cn.tile([P, 1], F32, name="eps_sb")
            nc.vector.memset(eps_sb, 1e-5)
            ones_sb = cn.tile([1, D], F32, name="ones_sb")
            nc.vector.memset(ones_sb, 1.0)

            QT_sb = cn.tile([P, CC, TCORE], F16, name="QT_sb")
            KT_sb = cn.tile([P, CC, T], F16, name="KT_sb")
            Vaug = cn.tile([P, CC, H, D + 1], F16, name="Vaug")
            nc.vector.memset(Vaug[:, :, :, D:D + 1], 1.0)
            OT_sb = cn.tile([P, CC, TCORE], F16, name="OT_sb")

            # ---------------- Q^T = Wq^T @ x^T  [C_out, 512] ----------------
            wq_sb = wp.tile([P, CC, C], F16, tag="w")
            nc.sync.dma_start(wq_sb, wq.rearrange("(c p) o -> p c o", p=P))
            for m in range(CC):
                ps = psm.tile([P, TCORE], F32, tag="psqk")
                for c in range(CC):
                    nc.tensor.matmul(ps, wq_sb[:, c, m * P:(m + 1) * P],
                                     xqT_sb[:, c, :],
                                     start=(c == 0), stop=(c == CC - 1))
                nc.scalar.activation(out=QT_sb[:, m, :], in_=ps, func=AF.Identity,
                                     bias=bq_sb[:, m:m + 1])

            # ---------------- K^T = Wk^T @ x^T  [C_out, 1024] ----------------
            wk_sb = wp.tile([P, CC, C], F16, tag="w")
            nc.sync.dma_start(wk_sb, wk.rearrange("(c p) o -> p c o", p=P))
            for m in range(CC):
                for n2 in range(2):
                    ps = psm.tile([P, TCORE], F32, tag="psqk")
                    for c in range(CC):
                        nc.tensor.matmul(ps, wk_sb[:, c, m * P:(m + 1) * P],
                                         xkvT_sb[:, c, n2 * TCORE:(n2 + 1) * TCORE],
                                         start=(c == 0), stop=(c == CC - 1))
                    nc.scalar.activation(out=KT_sb[:, m, n2 * TCORE:(n2 + 1) * TCORE],
                                         in_=ps, func=AF.Identity,
                                         bias=bk_sb[:, m:m + 1])

            # ---------------- V natural [kv, dims] + ones column ----------------
            wv_sb = wp.tile([P, CC, C], F16, tag="w")
            nc.sync.dma_start(wv_sb, wv.rearrange("(c p) o -> p c o", p=P))
            for kc in range(CC):
                for n2 in range(2):
                    ps = psm.tile([P, TCORE], F32, tag="psqk")
                    for c in range(CC):
                        nc.tensor.matmul(ps, xkvT_sb[:, c, kc * P:(kc + 1) * P],
                                         wv_sb[:, c, n2 * TCORE:(n2 + 1) * TCORE],
                                         start=(c == 0), stop=(c == CC - 1))
                    nc.vector.tensor_add(
                        out=Vaug[:, kc, n2 * 8:(n2 + 1) * 8, 0:D],
                        in0=ps.rearrange("p (h d) -> p h d", h=8),
                        in1=bv_rep[:, n2 * TCORE:(n2 + 1) * TCORE]
                        .rearrange("p (h d) -> p h d", h=8),
                    )

            # ---------------- attention + Wo + LN1, per query tile ----------------
            wo_sb = wp.tile([P, CC, C], F16, tag="w")
            nc.sync.dma_start(wo_sb, wo.rearrange("(c p) o -> p c o", p=P))
            for i in range(QTILES):
                nkv = 5 + i
                for h in range(H):
                    prow = (h % 2) * D
                    hc2 = h // 2
                    ps_s = pss.tile([P, 8 * P], F32, tag="ps_s")
                    for kvb in range(nkv):
                        nc.tensor.matmul(
                            ps_s[:, kvb * P:(kvb + 1) * P],
                            KT_sb[prow:prow + D, hc2, kvb * P:(kvb + 1) * P],
                            QT_sb[prow:prow + D, hc2, i * P:(i + 1) * P],
                            start=True, stop=True)
                    pT = ptp.tile([P, 8 * P], F16, tag="pT")
                    nc.scalar.activation(out=pT[:, :nkv * P], in_=ps_s[:, :nkv * P],
                                         func=AF.Exp, scale=0.125)
                    nc.vector.tensor_mul(
                        pT[:, :nkv * P].rearrange("p (b q) -> p b q", b=nkv),
                        pT[:, :nkv * P].rearrange("p (b q) -> p b q", b=nkv),
                        m01_sb[:, 0:nkv, i * P:(i + 1) * P])
                    ps_o = pso.tile([D + 1, P], F32, tag="ps_o")
                    for kvb in range(nkv):
                        nc.tensor.matmul(ps_o, Vaug[:, kvb, h, :],
                                         pT[:, kvb * P:(kvb + 1) * P],
                                         start=(kvb == 0), stop=(kvb == nkv - 1))
                    recip = wrk.tile([1, P], F32, tag="recip")
                    nc.vector.reciprocal(out=recip, in_=ps_o[D:D + 1, :])
                    ps_b = psb.tile([D, P], F32, tag="ps_b")
                    nc.tensor.matmul(ps_b, ones_sb, recip, start=True, stop=True)
                    nrm = wrk.tile([D, P], F32, tag="nrm")
                    nc.scalar.activation(out=nrm, in_=ps_b, func=AF.Copy)
                    nc.vector.tensor_mul(OT_sb[prow:prow + D, hc2, i * P:(i + 1) * P],
                                         ps_o[0:D, :], nrm)

                # Wo projection back to token-major + residual + LN1
                xn_sb = wrk.tile([P, C], F32, tag="xn_sb")
                nc.sync.dma_start(xn_sb, xn[i * P:(i + 1) * P, :])
                y_sb = wrk.tile([P, C], F32, tag="y_sb")
                for n2 in range(2):
                    ps_m = psm.tile([P, TCORE], F32, tag="psqk")
                    for c in range(CC):
                        nc.tensor.matmul(ps_m, OT_sb[:, c, i * P:(i + 1) * P],
                                         wo_sb[:, c, n2 * TCORE:(n2 + 1) * TCORE],
                                         start=(c == 0), stop=(c == CC - 1))
                    nc.vector.tensor_add(y_sb[:, n2 * TCORE:(n2 + 1) * TCORE], ps_m,
                                         xn_sb[:, n2 * TCORE:(n2 + 1) * TCORE])
                nc.vector.tensor_add(y_sb, y_sb, bo_rep)
                x1_sb = wrk.tile([P, C], F32, tag="x1_sb")
                _layernorm(nc, wrk, y_sb, g1_rep, bl1_rep, eps_sb, x1_sb, "1")
                nc.sync.dma_start(x1[i * P:(i + 1) * P, :], x1_sb)

    nc.finalize()
    return nc


def build_phase2():
    nc = bacc.Bacc(None)
    xgT = nc.declare_dram_parameter("xgT", [C, CAP], F16, isOutput=False)
    w1 = nc.declare_dram_parameter("w1", [C, HID], F16, isOutput=False)
    w2 = nc.declare_dram_parameter("w2", [HID, C], F16, isOutput=False)
    b1 = nc.declare_dram_parameter("b1", [HID], F32, isOutput=False)
    b2 = nc.declare_dram_parameter("b2", [C], F32, isOutput=False)
    gate = nc.declare_dram_parameter("gate", [CAP], F32, isOutput=False)
    contrib = nc.declare_dram_parameter("contrib", [CAP, C], F32, isOutput=True)

    with tile.TileContext(nc) as tc:
        with tc.tile_pool(name="cn", bufs=1) as cn, \
             tc.tile_pool(name="sl", bufs=2) as sl, \
             tc.tile_pool(name="ev", bufs=3) as ev, \
             tc.tile_pool(name="psh", bufs=2, space="PSUM") as psh, \
             tc.tile_pool(name="pso", bufs=2, space="PSUM") as pso:

            w1_sb = cn.tile([P, CC, HID], F16, name="w1_sb")
            nc.sync.dma_start(w1_sb, w1.rearrange("(c p) h -> p c h", p=P))
            w2_sb = cn.tile([P, HC, C], F16, name="w2_sb")
            nc.sync.dma_start(w2_sb, w2.rearrange("(h p) c -> p h c", p=P))
            b1_sb = cn.tile([P, HC], F32, name="b1_sb")
            nc.gpsimd.dma_start(b1_sb, b1.rearrange("(h p) -> p h", p=P))
            b2_rep = cn.tile([P, C], F32, name="b2_rep")
            nc.gpsimd.dma_start(b2_rep, _bcast_ap(b2[:]))
            gate_sb = cn.tile([P, CAP // P], F32, name="gate_sb")
            nc.gpsimd.dma_start(gate_sb, gate.rearrange("(m p) -> p m", p=P))

            for s in range(NSLICE):
                xg_sl = sl.tile([P, CC, CAP_SLICE], F16, tag="xg")
                nc.sync.dma_start(
                    xg_sl,
                    xgT.rearrange("(c p) t -> p c t", p=P)
                    [:, :, s * CAP_SLICE:(s + 1) * CAP_SLICE])
                hT_sl = sl.tile([P, HC, CAP_SLICE], F16, tag="hT")
                for hc in range(HC):
                    ps_h = psh.tile([P, CAP_SLICE], F32, tag="ps_h")
                    for c in range(CC):
                        nc.tensor.matmul(ps_h, w1_sb[:, c, hc * P:(hc + 1) * P],
                                         xg_sl[:, c, :],
                                         start=(c == 0), stop=(c == CC - 1))
                    nc.scalar.activation(out=hT_sl[:, hc, :], in_=ps_h,
                                         func=AF.Relu, bias=b1_sb[:, hc:hc + 1])
                for mc in range(CAP_SLICE // P):
                    gchunk = s * (CAP_SLICE // P) + mc
                    for n2 in range(2):
                        ps_o = pso.tile([P, TCORE], F32, tag="ps_o")
                        for hc in range(HC):
                            nc.tensor.matmul(
                                ps_o, hT_sl[:, hc, mc * P:(mc + 1) * P],
                                w2_sb[:, hc, n2 * TCORE:(n2 + 1) * TCORE],
                                start=(hc == 0), stop=(hc == HC - 1))
                        t_sb = ev.tile([P, TCORE], F32, tag="t_sb")
                        nc.vector.tensor_add(t_sb, ps_o,
                                             b2_rep[:, n2 * TCORE:(n2 + 1) * TCORE])
                        nc.vector.tensor_scalar_mul(
                            t_sb, t_sb, gate_sb[:, gchunk:gchunk + 1])
                        nc.sync.dma_start(
                            contrib[gchunk * P:(gchunk + 1) * P,
                                    n2 * TCORE:(n2 + 1) * TCORE], t_sb)

    nc.finalize()
    return nc


def build_phase3():
    nc = bacc.Bacc(None)
    y = nc.declare_dram_parameter("y", [TCORE, C], F32, isOutput=False)
    g2 = nc.declare_dram_parameter("g2", [C], F32, isOutput=False)
    bl2 = nc.declare_dram_parameter("bl2", [C], F32, isOutput=False)
    x2 = nc.declare_dram_parameter("x2", [TCORE, C], F32, isOutput=True)

    with tile.TileContext(nc) as tc:
        with tc.tile_pool(name="cn", bufs=1) as cn, \
             tc.tile_pool(name="wk3", bufs=3) as wrk:
            g_rep = cn.tile([P, C], F32, name="g_rep")
            nc.gpsimd.dma_start(g_rep, _bcast_ap(g2[:]))
            b_rep = cn.tile([P, C], F32, name="b_rep")
            nc.gpsimd.dma_start(b_rep, _bcast_ap(bl2[:]))
            eps_sb = cn.tile([P, 1], F32, name="eps_sb")
            nc.vector.memset(eps_sb, 1e-5)
            for i in range(QTILES):
                y_sb = wrk.tile([P, C], F32, tag="y_sb")
                nc.sync.dma_start(y_sb, y[i * P:(i + 1) * P, :])
                o_sb = wrk.tile([P, C], F32, tag="o_sb")
                _layernorm(nc, wrk, y_sb, g_rep, b_rep, eps_sb, o_sb, "3")
                nc.sync.dma_start(x2[i * P:(i + 1) * P, :], o_sb)

    nc.finalize()
    return nc


_PROGRAMS = {}
_DEBUG = {}


def _programs():
    if not _PROGRAMS:
        _PROGRAMS["p1"] = build_phase1()
        _PROGRAMS["p2"] = build_phase2()
        _PROGRAMS["p3"] = build_phase3()
    return _PROGRAMS


def kernel(**inputs):
    import time as _time
    inp = {k: np.asarray(v) for k, v in inputs.items()}
    x = inp["x"].astype(np.float32)
    amask = inp["attention_mask"]
    progs = _programs()

    w16 = {k: inp[k].astype(np.float16) for k in ("Wq", "Wk", "Wv", "Wo")}
    shared1 = dict(
        wq=w16["Wq"], wk=w16["Wk"], wv=w16["Wv"], wo=w16["Wo"],
        bq=inp["bq"].astype(np.float32), bk=inp["bk"].astype(np.float32),
        bv=inp["bv"].astype(np.float32), bo=inp["bo"].astype(np.float32),
        g1=inp["ln1_g"].astype(np.float32), bl1=inp["ln1_b"].astype(np.float32),
    )

    in_maps1 = []
    kv_idx = np.arange(T)
    for core in range(NCORES):
        b, r0 = core // 2, TCORE * (core % 2)
        xb = x[b]
        q_idx = r0 + np.arange(TCORE)
        m01 = ((kv_idx[:, None] <= q_idx[None, :])
               & (amask[b][:, None] > 0)).astype(np.float16)
        in_maps1.append(dict(
            xqT=np.ascontiguousarray(xb[r0:r0 + TCORE].T).astype(np.float16),
            xkvT=np.ascontiguousarray(xb.T).astype(np.float16),
            xn=np.ascontiguousarray(xb[r0:r0 + TCORE]),
            m01=m01, **shared1))
    _t = _time.perf_counter()
    res1 = run_bass_kernel_spmd(progs["p1"], in_maps1, list(range(NCORES)))
    _DEBUG["t_p1"] = _time.perf_counter() - _t
    x1 = np.concatenate([res1.results[c]["x1"] for c in range(NCORES)], axis=0)

    # ---------------- host routing (exact fp32, reference semantics) ---------
    logits = (x1 @ inp["Wr"].astype(np.float32)
              + inp["br"].astype(np.float32))              # [4096, E]
    order = np.argsort(-logits, axis=-1, kind="stable")
    top2 = order[:, :TOPK]
    emask = np.zeros((B * T, E), dtype=bool)
    np.put_along_axis(emask, top2, True, axis=-1)
    mx = logits.max(-1, keepdims=True)
    pe = np.exp(logits - mx)
    probs = pe / pe.sum(-1, keepdims=True)
    gl = np.where(emask, logits, np.float32(-1e9))
    mg = gl.max(-1, keepdims=True)
    ge = np.exp(gl - mg)
    gates = ge / ge.sum(-1, keepdims=True)
    importance = probs.mean(0)
    load = emask.astype(np.float32).mean(0)
    lb_loss = np.float32(E) * np.float32(np.sum(importance * load))

    # ---------------- expert-parallel MLP ------------------------------------
    W1, W2 = inp["W1"], inp["W2"]
    b1, b2 = inp["b1"].astype(np.float32), inp["b2"].astype(np.float32)
    idx_e, in_maps2 = [], []
    for e in range(E):
        idx = np.nonzero(emask[:, e])[0]
        idx_e.append(idx)
        n = min(len(idx), CAP)
        xg = np.zeros((CAP, C), dtype=np.float16)
        xg[:n] = x1[idx[:n]].astype(np.float16)
        gv = np.zeros(CAP, dtype=np.float32)
        gv[:n] = gates[idx[:n], e]
        in_maps2.append(dict(
            xgT=np.ascontiguousarray(xg.T),
            w1=W1[e].astype(np.float16), w2=W2[e].astype(np.float16),
            b1=b1[e], b2=b2[e], gate=gv))
    _t = _time.perf_counter()
    res2 = run_bass_kernel_spmd(progs["p2"], in_maps2, list(range(NCORES)))
    _DEBUG["t_p2"] = _time.perf_counter() - _t
    _DEBUG["top2"] = top2

    moe = np.zeros((B * T, C), dtype=np.float32)
    for e in range(E):
        idx = idx_e[e]
        n = min(len(idx), CAP)
        moe[idx[:n]] += res2.results[e]["contrib"][:n]
        if len(idx) > CAP:  # capacity overflow fallback (exact, host fp32)
            for t in idx[CAP:]:
                h = np.maximum(x1[t] @ W1[e] + b1[e], 0.0)
                moe[t] += gates[t, e] * (h @ W2[e] + b2[e])

    y = x1 + moe

    # ---------------- LN2 ------------------------------------------------------
    shared3 = dict(g2=inp["ln2_g"].astype(np.float32),
                   bl2=inp["ln2_b"].astype(np.float32))
    in_maps3 = [dict(y=y[c * TCORE:(c + 1) * TCORE], **shared3)
                for c in range(NCORES)]
    _t = _time.perf_counter()
    res3 = run_bass_kernel_spmd(progs["p3"], in_maps3, list(range(NCORES)))
    _DEBUG["t_p3"] = _time.perf_counter() - _t
    x2 = np.concatenate([res3.results[c]["x2"] for c in range(NCORES)],
                        axis=0).reshape(B, T, C)
    return (x2, lb_loss)


# revision 17
# speedup vs baseline: 1.0057x; 1.0057x over previous
"""Trainium2 Bass kernel for nn_Block_25323127177341 (moe_routing).

Distribution (8 NeuronCores):
  Phase 1 (data-parallel, 512 tokens/core): fused QKV projections (fp16 PE,
    transposed-activation layout), causal multi-head attention with fp32-PSUM
    softmax (no max-subtraction; scores are small), output projection,
    residual + LayerNorm1 -> x1.
  Host: router logits / top-2 / gates / load-balance loss in exact fp32;
    all-to-all token routing done host-side (full-IO contract).
  Phase 2 (expert-parallel, 1 expert/core): gathered tokens through the
    expert MLP (fp16 PE, fp32 accum), relu, gate scaling -> contributions.
  Host: scatter-add combine + residual.
  Phase 3 (data-parallel): LayerNorm2 -> x2.

`repeat` on the builders wraps the whole body in an on-device For_i loop;
used only by the timing harness (wall-clock slope = per-iteration HW time).
"""

import contextlib
import sys

import numpy as np

try:
    import concourse.bass as bass  # noqa: F401
except ImportError:  # pragma: no cover
    for _p in ("/opt/trn_rl_repo", "/root/.axon_site/_ro/trn_rl_repo"):
        if _p not in sys.path:
            sys.path.insert(0, _p)
    import concourse.bass as bass  # noqa: F401

import concourse.mybir as mybir
import concourse.tile as tile
from concourse import bacc
from concourse.bass_utils import run_bass_kernel_spmd

F16 = mybir.dt.float16
F32 = mybir.dt.float32
AF = mybir.ActivationFunctionType
ALU = mybir.AluOpType

P = 128
B, T, C, H, E, TOPK = 4, 1024, 1024, 16, 8, 2
D = C // H            # 64 head dim
HID = 4 * C           # 4096
NCORES = 8
TCORE = T // 2        # 512 tokens per core in phase 1/3
CC = C // P           # 8 C-chunks
QTILES = TCORE // P   # 4 query tiles per core
CAP = 1152            # expert capacity (max observed count 1086)
SLICES = (256, 256, 256, 256, 128)   # token slices processed per pass
HC = HID // P         # 32 hidden chunks


def _bcast_ap(vec_ap, parts=P):
    """DRAM AP for a 1-D vector replicated across `parts` partitions."""
    return bass.AP(
        tensor=vec_ap.tensor,
        offset=vec_ap.offset,
        ap=[[0, parts]] + [list(d) for d in vec_ap.ap],
    )


def _layernorm(nc, pool, y_sb, g_rep, b_rep, eps_sb, out_sb, tag):
    """out = LN(y) * g + b over the free dim (1024), tokens on partitions."""
    stats = pool.tile([P, 2, 6], F32, tag=f"lnst{tag}")
    yr = y_sb.rearrange("p (s d) -> p s d", s=2)
    for s in range(2):
        nc.vector.bn_stats(out=stats[:, s, :], in_=yr[:, s, :])
    mv = pool.tile([P, 2], F32, tag=f"lnmv{tag}")
    nc.vector.bn_aggr(out=mv, in_=stats)
    rstd = pool.tile([P, 1], F32, tag=f"lnrs{tag}")
    nc.scalar.activation(out=rstd, in_=mv[:, 1:2], func=AF.Sqrt, bias=eps_sb)
    nc.vector.reciprocal(out=rstd, in_=rstd)
    nc.vector.tensor_scalar(
        out=out_sb, in0=y_sb, scalar1=mv[:, 0:1], scalar2=rstd,
        op0=ALU.subtract, op1=ALU.mult,
    )
    nc.vector.tensor_mul(out_sb, out_sb, g_rep)
    nc.vector.tensor_add(out_sb, out_sb, b_rep)


def build_phase1(repeat=1):
    nc = bacc.Bacc(None)
    xqT = nc.declare_dram_parameter("xqT", [C, TCORE], F16, isOutput=False)
    xkvT = nc.declare_dram_parameter("xkvT", [C, T], F16, isOutput=False)
    xn = nc.declare_dram_parameter("xn", [TCORE, C], F32, isOutput=False)
    m01 = nc.declare_dram_parameter("m01", [T, TCORE], F16, isOutput=False)
    wq = nc.declare_dram_parameter("wq", [C, C], F16, isOutput=False)
    wk = nc.declare_dram_parameter("wk", [C, C], F16, isOutput=False)
    wv = nc.declare_dram_parameter("wv", [C, C], F16, isOutput=False)
    wo = nc.declare_dram_parameter("wo", [C, C], F16, isOutput=False)
    bq = nc.declare_dram_parameter("bq", [C], F32, isOutput=False)
    bk = nc.declare_dram_parameter("bk", [C], F32, isOutput=False)
    bv = nc.declare_dram_parameter("bv", [C], F32, isOutput=False)
    bo = nc.declare_dram_parameter("bo", [C], F32, isOutput=False)
    g1 = nc.declare_dram_parameter("g1", [C], F32, isOutput=False)
    bl1 = nc.declare_dram_parameter("bl1", [C], F32, isOutput=False)
    x1 = nc.declare_dram_parameter("x1", [TCORE, C], F32, isOutput=True)

    with tile.TileContext(nc) as tc:
        with tc.tile_pool(name="cn", bufs=1) as cn, \
             tc.tile_pool(name="wp", bufs=2) as wp, \
             tc.tile_pool(name="wk2", bufs=3) as wrk, \
             tc.tile_pool(name="ptp", bufs=3) as ptp, \
             tc.tile_pool(name="pss", bufs=2, space="PSUM") as pss, \
             tc.tile_pool(name="pso", bufs=2, space="PSUM") as pso, \
             tc.tile_pool(name="psb", bufs=1, space="PSUM") as psb, \
             tc.tile_pool(name="psm", bufs=1, space="PSUM") as psm, \
             (tc.For_i(0, repeat, 1) if repeat > 1
              else contextlib.nullcontext()):

            # ---------------- constant / input loads ----------------
            xqT_sb = cn.tile([P, CC, TCORE], F16, name="xqT_sb")
            nc.sync.dma_start(xqT_sb, xqT.rearrange("(c p) q -> p c q", p=P))
            xkvT_sb = cn.tile([P, CC, T], F16, name="xkvT_sb")
            nc.sync.dma_start(xkvT_sb, xkvT.rearrange("(c p) t -> p c t", p=P))
            m01_sb = cn.tile([P, CC, TCORE], F16, name="m01_sb")
            nc.sync.dma_start(m01_sb, m01.rearrange("(c p) q -> p c q", p=P))

            bq_sb = cn.tile([P, CC], F32, name="bq_sb")
            nc.gpsimd.dma_start(bq_sb, bq.rearrange("(o p) -> p o", p=P))
            bk_sb = cn.tile([P, CC], F32, name="bk_sb")
            nc.gpsimd.dma_start(bk_sb, bk.rearrange("(o p) -> p o", p=P))
            bv_rep = cn.tile([P, C], F32, name="bv_rep")
            nc.gpsimd.dma_start(bv_rep, _bcast_ap(bv[:]))
            bo_rep = cn.tile([P, C], F32, name="bo_rep")
            nc.gpsimd.dma_start(bo_rep, _bcast_ap(bo[:]))
            g1_rep = cn.tile([P, C], F32, name="g1_rep")
            nc.gpsimd.dma_start(g1_rep, _bcast_ap(g1[:]))
            bl1_rep = cn.tile([P, C], F32, name="bl1_rep")
            nc.gpsimd.dma_start(bl1_rep, _bcast_ap(bl1[:]))
            eps_sb = cn.tile([P, 1], F32, name="eps_sb")
            nc.vector.memset(eps_sb, 1e-5)
            ones_sb = cn.tile([1, D], F32, name="ones_sb")
            nc.vector.memset(ones_sb, 1.0)

            QT_sb = cn.tile([P, CC, TCORE], F16, name="QT_sb")
            KT_sb = cn.tile([P, CC, T], F16, name="KT_sb")
            Vaug = cn.tile([P, CC, H, D + 1], F16, name="Vaug")
            nc.vector.memset(Vaug[:, :, :, D:D + 1], 1.0)
            OT_sb = cn.tile([P, CC, TCORE], F16, name="OT_sb")

            # ------------- Q^T = Wq^T @ x^T  [C_out, 512] -------------
            wq_sb = wp.tile([P, CC, C], F16, tag="w")
            nc.sync.dma_start(wq_sb, wq.rearrange("(c p) o -> p c o", p=P))
            for m in range(CC):
                ps = psm.tile([P, TCORE], F32, tag="psqk")
                for c in range(CC):
                    nc.tensor.matmul(ps, wq_sb[:, c, m * P:(m + 1) * P],
                                     xqT_sb[:, c, :],
                                     start=(c == 0), stop=(c == CC - 1))
                nc.scalar.activation(out=QT_sb[:, m, :], in_=ps,
                                     func=AF.Identity, bias=bq_sb[:, m:m + 1])

            # ------------- K^T = Wk^T @ x^T  [C_out, 1024] -------------
            wk_sb = wp.tile([P, CC, C], F16, tag="w")
            nc.sync.dma_start(wk_sb, wk.rearrange("(c p) o -> p c o", p=P))
            for m in range(CC):
                for n2 in range(2):
                    ps = psm.tile([P, TCORE], F32, tag="psqk")
                    for c in range(CC):
                        nc.tensor.matmul(
                            ps, wk_sb[:, c, m * P:(m + 1) * P],
                            xkvT_sb[:, c, n2 * TCORE:(n2 + 1) * TCORE],
                            start=(c == 0), stop=(c == CC - 1))
                    nc.scalar.activation(
                        out=KT_sb[:, m, n2 * TCORE:(n2 + 1) * TCORE],
                        in_=ps, func=AF.Identity, bias=bk_sb[:, m:m + 1])

            # ------------- V natural [kv, dims] + ones column -------------
            wv_sb = wp.tile([P, CC, C], F16, tag="w")
            nc.sync.dma_start(wv_sb, wv.rearrange("(c p) o -> p c o", p=P))
            for kc in range(CC):
                for n2 in range(2):
                    ps = psm.tile([P, TCORE], F32, tag="psqk")
                    for c in range(CC):
                        nc.tensor.matmul(
                            ps, xkvT_sb[:, c, kc * P:(kc + 1) * P],
                            wv_sb[:, c, n2 * TCORE:(n2 + 1) * TCORE],
                            start=(c == 0), stop=(c == CC - 1))
                    nc.vector.tensor_add(
                        out=Vaug[:, kc, n2 * 8:(n2 + 1) * 8, 0:D],
                        in0=ps.rearrange("p (h d) -> p h d", h=8),
                        in1=bv_rep[:, n2 * TCORE:(n2 + 1) * TCORE]
                        .rearrange("p (h d) -> p h d", h=8),
                    )

            # ------- attention + Wo + LN1, per query-tile PAIR -------
            # Query tiles are processed two at a time (N=256 on the PE) to
            # halve instruction counts; score PSUM is split into half-tiles
            # (<=4 kv blocks each) so exp ping-pongs with the score matmuls.
            wo_sb = wp.tile([P, CC, C], F16, tag="w")
            nc.sync.dma_start(wo_sb, wo.rearrange("(c p) o -> p c o", p=P))
            Q2 = 2 * P  # 256 queries per pair
            for pair in range(QTILES // 2):
                nkv = 6 + 2 * pair  # kv blocks covering both tiles (masked)
                qs = pair * Q2
                for h in range(H):
                    prow = (h % 2) * D
                    hc2 = h // 2
                    pT = ptp.tile([P, 8, Q2], F16, tag="pT")
                    for half in range(2):
                        b0 = half * 4
                        nb = min(4, nkv - b0)
                        ps_s = pss.tile([P, 4, Q2], F32, tag="ps_s")
                        for j in range(nb):
                            nc.tensor.matmul(
                                ps_s[:, j, :],
                                KT_sb[prow:prow + D, hc2,
                                      (b0 + j) * P:(b0 + j + 1) * P],
                                QT_sb[prow:prow + D, hc2, qs:qs + Q2],
                                start=True, stop=True)
                        nc.scalar.activation(
                            out=pT[:, b0:b0 + nb, :],
                            in_=ps_s[:, 0:nb, :], func=AF.Exp, scale=0.125)
                        # blocks 0..2*pair-1 are fully valid on every core;
                        # only the rest need the mask multiply
                        mskip = min(2 * pair, b0 + nb)
                        m0 = max(b0, mskip)
                        if m0 < b0 + nb:
                            nc.vector.tensor_mul(
                                pT[:, m0:b0 + nb, :],
                                pT[:, m0:b0 + nb, :],
                                m01_sb[:, m0:b0 + nb, qs:qs + Q2])
                    ps_o = pso.tile([D + 1, Q2], F32, tag="ps_o")
                    for kvb in range(nkv):
                        nc.tensor.matmul(ps_o, Vaug[:, kvb, h, :],
                                         pT[:, kvb, :],
                                         start=(kvb == 0), stop=(kvb == nkv - 1))
                    recip = wrk.tile([1, Q2], F32, tag="recip")
                    nc.vector.reciprocal(out=recip, in_=ps_o[D:D + 1, :])
                    ps_b = psb.tile([D, Q2], F32, tag="ps_b")
                    nc.tensor.matmul(ps_b, ones_sb, recip, start=True, stop=True)
                    nrm = wrk.tile([D, Q2], F32, tag="nrm")
                    nc.scalar.activation(out=nrm, in_=ps_b, func=AF.Copy)
                    nc.vector.tensor_mul(
                        OT_sb[prow:prow + D, hc2, qs:qs + Q2],
                        ps_o[0:D, :], nrm)

                # Wo projection back to token-major + residual + LN1
                for it in range(2):
                    i = pair * 2 + it
                    xn_sb = wrk.tile([P, C], F32, tag="xn_sb")
                    nc.sync.dma_start(xn_sb, xn[i * P:(i + 1) * P, :])
                    y_sb = wrk.tile([P, C], F32, tag="y_sb")
                    for n2 in range(2):
                        ps_m = psm.tile([P, TCORE], F32, tag="psqk")
                        for c in range(CC):
                            nc.tensor.matmul(
                                ps_m, OT_sb[:, c, i * P:(i + 1) * P],
                                wo_sb[:, c, n2 * TCORE:(n2 + 1) * TCORE],
                                start=(c == 0), stop=(c == CC - 1))
                        nc.vector.tensor_add(
                            y_sb[:, n2 * TCORE:(n2 + 1) * TCORE], ps_m,
                            xn_sb[:, n2 * TCORE:(n2 + 1) * TCORE])
                    nc.vector.tensor_add(y_sb, y_sb, bo_rep)
                    x1_sb = wrk.tile([P, C], F32, tag="x1_sb")
                    _layernorm(nc, wrk, y_sb, g1_rep, bl1_rep, eps_sb, x1_sb, "1")
                    nc.sync.dma_start(x1[i * P:(i + 1) * P, :], x1_sb)

    nc.finalize()
    return nc


def build_phase2(repeat=1):
    nc = bacc.Bacc(None)
    xgT = nc.declare_dram_parameter("xgT", [C, CAP], F16, isOutput=False)
    w1 = nc.declare_dram_parameter("w1", [C, HID], F16, isOutput=False)
    w2 = nc.declare_dram_parameter("w2", [HID, C], F16, isOutput=False)
    b1 = nc.declare_dram_parameter("b1", [HID], F32, isOutput=False)
    gate = nc.declare_dram_parameter("gate", [CAP], F32, isOutput=False)
    contrib = nc.declare_dram_parameter("contrib", [CAP, C], F32, isOutput=True)

    with tile.TileContext(nc) as tc:
        with tc.tile_pool(name="cn", bufs=1) as cn, \
             tc.tile_pool(name="sl", bufs=2) as sl, \
             tc.tile_pool(name="ev", bufs=3) as ev, \
             tc.tile_pool(name="psh", bufs=2, space="PSUM") as psh, \
             tc.tile_pool(name="pso", bufs=2, space="PSUM") as pso, \
             (tc.For_i(0, repeat, 1) if repeat > 1
              else contextlib.nullcontext()):

            # chunked weight loads so the first matmuls start early
            w1_sb = cn.tile([P, CC, HID], F16, name="w1_sb")
            w1r = w1.rearrange("(c p) h -> p c h", p=P)
            for c in range(CC):
                nc.sync.dma_start(w1_sb[:, c, :], w1r[:, c, :])
            w2_sb = cn.tile([P, HC, C], F16, name="w2_sb")
            w2r = w2.rearrange("(h p) c -> p h c", p=P)
            for hq in range(4):
                nc.sync.dma_start(w2_sb[:, hq * 8:(hq + 1) * 8, :],
                                  w2r[:, hq * 8:(hq + 1) * 8, :])
            b1_sb = cn.tile([P, HC], F32, name="b1_sb")
            nc.gpsimd.dma_start(b1_sb, b1.rearrange("(h p) -> p h", p=P))
            gate_sb = cn.tile([P, CAP // P], F32, name="gate_sb")
            nc.gpsimd.dma_start(gate_sb, gate.rearrange("(m p) -> p m", p=P))

            xgTr = xgT.rearrange("(c p) t -> p c t", p=P)
            t0 = 0
            for s, slc in enumerate(SLICES):
                xg_sl = sl.tile([P, CC, slc], F16, tag="xg")
                nc.sync.dma_start(xg_sl, xgTr[:, :, t0:t0 + slc])
                hT_sl = sl.tile([P, HC, slc], F16, tag="hT")
                for hc in range(HC):
                    ps_h = psh.tile([P, slc], F32, tag="ps_h")
                    for c in range(CC):
                        nc.tensor.matmul(ps_h, w1_sb[:, c, hc * P:(hc + 1) * P],
                                         xg_sl[:, c, :],
                                         start=(c == 0), stop=(c == CC - 1))
                    nc.scalar.activation(out=hT_sl[:, hc, :], in_=ps_h,
                                         func=AF.Relu, bias=b1_sb[:, hc:hc + 1])
                for mc in range(slc // P):
                    gchunk = t0 // P + mc
                    for n2 in range(2):
                        ps_o = pso.tile([P, TCORE], F32, tag="ps_o")
                        for hc in range(HC):
                            nc.tensor.matmul(
                                ps_o, hT_sl[:, hc, mc * P:(mc + 1) * P],
                                w2_sb[:, hc, n2 * TCORE:(n2 + 1) * TCORE],
                                start=(hc == 0), stop=(hc == HC - 1))
                        t_sb = ev.tile([P, TCORE], F32, tag="t_sb")
                        nc.scalar.activation(
                            out=t_sb, in_=ps_o, func=AF.Identity,
                            scale=gate_sb[:, gchunk:gchunk + 1])
                        nc.sync.dma_start(
                            contrib[gchunk * P:(gchunk + 1) * P,
                                    n2 * TCORE:(n2 + 1) * TCORE], t_sb)
                t0 += slc

    nc.finalize()
    return nc


def build_phase3(repeat=1):
    nc = bacc.Bacc(None)
    y = nc.declare_dram_parameter("y", [TCORE, C], F32, isOutput=False)
    g2 = nc.declare_dram_parameter("g2", [C], F32, isOutput=False)
    bl2 = nc.declare_dram_parameter("bl2", [C], F32, isOutput=False)
    x2 = nc.declare_dram_parameter("x2", [TCORE, C], F32, isOutput=True)

    with tile.TileContext(nc) as tc:
        with tc.tile_pool(name="cn", bufs=1) as cn, \
             tc.tile_pool(name="wk3", bufs=3) as wrk, \
             (tc.For_i(0, repeat, 1) if repeat > 1
              else contextlib.nullcontext()):
            g_rep = cn.tile([P, C], F32, name="g_rep")
            nc.gpsimd.dma_start(g_rep, _bcast_ap(g2[:]))
            b_rep = cn.tile([P, C], F32, name="b_rep")
            nc.gpsimd.dma_start(b_rep, _bcast_ap(bl2[:]))
            eps_sb = cn.tile([P, 1], F32, name="eps_sb")
            nc.vector.memset(eps_sb, 1e-5)
            for i in range(QTILES):
                y_sb = wrk.tile([P, C], F32, tag="y_sb")
                nc.sync.dma_start(y_sb, y[i * P:(i + 1) * P, :])
                o_sb = wrk.tile([P, C], F32, tag="o_sb")
                _layernorm(nc, wrk, y_sb, g_rep, b_rep, eps_sb, o_sb, "3")
                nc.sync.dma_start(x2[i * P:(i + 1) * P, :], o_sb)

    nc.finalize()
    return nc


_PROGRAMS = {}
_DEBUG = {}


def _programs():
    if not _PROGRAMS:
        _PROGRAMS["p1"] = build_phase1()
        _PROGRAMS["p2"] = build_phase2()
        _PROGRAMS["p3"] = build_phase3()
    return _PROGRAMS


def kernel(**inputs):
    import time as _time
    inp = {k: np.asarray(v) for k, v in inputs.items()}
    x = inp["x"].astype(np.float32)
    amask = inp["attention_mask"]
    progs = _programs()

    w16 = {k: inp[k].astype(np.float16) for k in ("Wq", "Wk", "Wv", "Wo")}
    shared1 = dict(
        wq=w16["Wq"], wk=w16["Wk"], wv=w16["Wv"], wo=w16["Wo"],
        bq=inp["bq"].astype(np.float32), bk=inp["bk"].astype(np.float32),
        bv=inp["bv"].astype(np.float32), bo=inp["bo"].astype(np.float32),
        g1=inp["ln1_g"].astype(np.float32), bl1=inp["ln1_b"].astype(np.float32),
    )

    in_maps1 = []
    kv_idx = np.arange(T)
    for core in range(NCORES):
        b, r0 = core // 2, TCORE * (core % 2)
        xb = x[b]
        q_idx = r0 + np.arange(TCORE)
        m01 = ((kv_idx[:, None] <= q_idx[None, :])
               & (amask[b][:, None] > 0)).astype(np.float16)
        in_maps1.append(dict(
            xqT=np.ascontiguousarray(xb[r0:r0 + TCORE].T).astype(np.float16),
            xkvT=np.ascontiguousarray(xb.T).astype(np.float16),
            xn=np.ascontiguousarray(xb[r0:r0 + TCORE]),
            m01=m01, **shared1))
    _t = _time.perf_counter()
    res1 = run_bass_kernel_spmd(progs["p1"], in_maps1, list(range(NCORES)))
    _DEBUG["t_p1"] = _time.perf_counter() - _t
    x1 = np.concatenate([res1.results[c]["x1"] for c in range(NCORES)], axis=0)

    # ---------------- host routing (exact fp32, reference semantics) ---------
    logits = (x1 @ inp["Wr"].astype(np.float32)
              + inp["br"].astype(np.float32))              # [4096, E]
    order = np.argsort(-logits, axis=-1, kind="stable")
    top2 = order[:, :TOPK]
    emask = np.zeros((B * T, E), dtype=bool)
    np.put_along_axis(emask, top2, True, axis=-1)
    mx = logits.max(-1, keepdims=True)
    pe = np.exp(logits - mx)
    probs = pe / pe.sum(-1, keepdims=True)
    gl = np.where(emask, logits, np.float32(-1e9))
    mg = gl.max(-1, keepdims=True)
    ge = np.exp(gl - mg)
    gates = ge / ge.sum(-1, keepdims=True)
    importance = probs.mean(0)
    load = emask.astype(np.float32).mean(0)
    lb_loss = np.float32(E) * np.float32(np.sum(importance * load))

    # ---------------- expert-parallel MLP ------------------------------------
    W1, W2 = inp["W1"], inp["W2"]
    b1, b2 = inp["b1"].astype(np.float32), inp["b2"].astype(np.float32)
    idx_e, in_maps2 = [], []
    for e in range(E):
        idx = np.nonzero(emask[:, e])[0]
        idx_e.append(idx)
        n = min(len(idx), CAP)
        xg = np.zeros((CAP, C), dtype=np.float16)
        xg[:n] = x1[idx[:n]].astype(np.float16)
        gv = np.zeros(CAP, dtype=np.float32)
        gv[:n] = gates[idx[:n], e]
        in_maps2.append(dict(
            xgT=np.ascontiguousarray(xg.T),
            w1=W1[e].astype(np.float16), w2=W2[e].astype(np.float16),
            b1=b1[e], gate=gv))
    _t = _time.perf_counter()
    res2 = run_bass_kernel_spmd(progs["p2"], in_maps2, list(range(NCORES)))
    _DEBUG["t_p2"] = _time.perf_counter() - _t
    _DEBUG["top2"] = top2

    # gate-weighted b2 bias is a rank-2 host add: sum_e gates[t,e] * b2[e]
    moe = (gates @ b2).astype(np.float32)
    for e in range(E):
        idx = idx_e[e]
        n = min(len(idx), CAP)
        moe[idx[:n]] += res2.results[e]["contrib"][:n]
        if len(idx) > CAP:  # capacity overflow fallback (exact, host fp32)
            for t in idx[CAP:]:
                hh = np.maximum(x1[t] @ W1[e] + b1[e], 0.0)
                moe[t] += gates[t, e] * (hh @ W2[e])

    y = x1 + moe

    # ---------------- LN2 -----------------------------------------------------
    shared3 = dict(g2=inp["ln2_g"].astype(np.float32),
                   bl2=inp["ln2_b"].astype(np.float32))
    in_maps3 = [dict(y=y[c * TCORE:(c + 1) * TCORE], **shared3)
                for c in range(NCORES)]
    _t = _time.perf_counter()
    res3 = run_bass_kernel_spmd(progs["p3"], in_maps3, list(range(NCORES)))
    _DEBUG["t_p3"] = _time.perf_counter() - _t
    x2 = np.concatenate([res3.results[c]["x2"] for c in range(NCORES)],
                        axis=0).reshape(B, T, C)
    return (x2, lb_loss)
